# revision 3
# baseline (speedup 1.0000x reference)
"""CARAFE content-aware upsampling on 8 Trainium2 NeuronCores.

Strategy (data parallel, hint-compliant):
  8 cores = 4 batch images x 2 row-halves (32 low-res rows each, +2-row halo).
  Per core, fully fused pipeline in SBUF:
    A) y_down = conv1x1(x, w_down)+b_down        (PE, K=256 in 2 chunks)
    Z) zT = (w_out . x) transposed               (PE produces [col, ch] directly)
    B) enc = conv3x3(y_down, w_enc)              (PE, 9 shifted accum matmuls)
    C) mask = softmax over 25 taps (4 groups)    (PE transpose+group-sums via an
       augmented selector matmul, DVE reciprocal + normalize)
    D) out = sum_k zT[window] * mask  + b_out    (PE: per-row banded matmuls;
       banded mask matrix built by a DRAM-roundtrip diagonal scatter DMA)
  The final 1x1 conv (w_out) is folded BEFORE reassembly (z-trick): conv and
  reassembly commute since both are linear; this runs the big conv at low res
  and skips materializing the upsampled intermediate.

Layouts:
  xs     [256, 36, 68]  zero-padded shard (rows h0-2..h1+2, cols -2..65)
  zT     [68, 36, 256]  col-on-partition transpose of z = w_out . x
  B_h    [68, 1280]     banded masks: B[w+j, w*20 + i*4 + p] = mask[h,w,i,j,p]
  out    [256, 64, 128] hi-res shard
"""

import sys
import functools
import numpy as np
from contextlib import ExitStack

for _p in ("/opt/trn_rl_repo",):
    if _p not in sys.path:
        sys.path.insert(0, _p)

import concourse.bass as bass
import concourse.bacc as bacc
import concourse.mybir as mybir
import concourse.tile as tile
from concourse.bass_utils import run_bass_kernel_spmd

NCORES = 8
FP = mybir.dt.float32
AF = mybir.ActivationFunctionType
ALU = mybir.AluOpType


def _ap(base, offset_delta, dims):
    return bass.AP(tensor=base.tensor, offset=base.offset + offset_delta, ap=dims)


@functools.lru_cache(maxsize=1)
def _build():
    nc = bacc.Bacc("TRN2", target_bir_lowering=False, debug=False, num_devices=NCORES)

    xs_d = nc.declare_dram_parameter("xs", [256, 36, 68], FP, isOutput=False)
    wdt_d = nc.declare_dram_parameter("wdt", [256, 128], FP, isOutput=False)
    wet_d = nc.declare_dram_parameter("wet", [128, 9, 100], FP, isOutput=False)
    wot_d = nc.declare_dram_parameter("wot", [256, 256], FP, isOutput=False)
    bd_d = nc.declare_dram_parameter("bd", [128, 1], FP, isOutput=False)
    be_d = nc.declare_dram_parameter("be", [100, 1], FP, isOutput=False)
    bo_d = nc.declare_dram_parameter("bo", [256, 1], FP, isOutput=False)
    saug_d = nc.declare_dram_parameter("saug", [100, 104], FP, isOutput=False)
    edge_d = nc.declare_dram_parameter("edge", [1, 2], FP, isOutput=False)
    out_d = nc.declare_dram_parameter("out", [256, 64, 128], FP, isOutput=True)

    with tile.TileContext(nc) as tc:
        with ExitStack() as ctx:
            const = ctx.enter_context(tc.tile_pool(name="const", bufs=1))
            big = ctx.enter_context(tc.tile_pool(name="big", bufs=1))
            bpool = ctx.enter_context(tc.tile_pool(name="bpool", bufs=1))
            opool = ctx.enter_context(tc.tile_pool(name="opool", bufs=3))
            dpool = ctx.enter_context(tc.tile_pool(name="dpool", bufs=1, space="DRAM"))
            psAZ = ctx.enter_context(tc.tile_pool(name="psAZ", bufs=2, space="PSUM"))
            psB = ctx.enter_context(tc.tile_pool(name="psB", bufs=2, space="PSUM"))
            psCD = ctx.enter_context(tc.tile_pool(name="psCD", bufs=2, space="PSUM"))

            # ---- loads ----
            xa = big.tile([128, 36, 68], FP)
            xb = big.tile([128, 36, 68], FP)
            nc.sync.dma_start(out=xa[:], in_=xs_d[0:128])
            nc.sync.dma_start(out=xb[:], in_=xs_d[128:256])
            wdt = const.tile([128, 2, 128], FP)
            nc.sync.dma_start(out=wdt[:, 0, :], in_=wdt_d[0:128])
            nc.sync.dma_start(out=wdt[:, 1, :], in_=wdt_d[128:256])
            wet = const.tile([128, 9, 100], FP)
            nc.sync.dma_start(out=wet[:], in_=wet_d[:])
            wot = const.tile([128, 2, 256], FP)
            nc.sync.dma_start(out=wot[:, 0, :], in_=wot_d[0:128])
            nc.sync.dma_start(out=wot[:, 1, :], in_=wot_d[128:256])
            bd = const.tile([128, 1], FP)
            nc.sync.dma_start(out=bd[:], in_=bd_d[:])
            be = const.tile([100, 1], FP)
            nc.sync.dma_start(out=be[:], in_=be_d[:])
            bo = const.tile([128, 2], FP)
            nc.sync.dma_start(out=bo[:, 0:1], in_=bo_d[0:128])
            nc.sync.dma_start(out=bo[:, 1:2], in_=bo_d[128:256])
            saug = const.tile([100, 104], FP)
            nc.sync.dma_start(out=saug[:], in_=saug_d[:])
            edge = const.tile([128, 2], FP)
            nc.sync.dma_start(
                out=edge[:],
                in_=bass.AP(tensor=edge_d, offset=0, ap=[[0, 128], [1, 2]]),
            )

            ydown = big.tile([128, 34, 66], FP)
            zt = big.tile([68, 36, 256], FP)
            expv = big.tile([100, 32, 64], FP)
            maskv = big.tile([128, 16, 100], FP)
            inv = big.tile([128, 16, 4], FP)

            # DRAM staging for the banded-mask scatter (3 rotating slots).
            bstage = []
            for i in range(3):
                st = dpool.tile([68, 1280], FP, tag=f"st{i}", name=f"bstage{i}")
                bstage.append(st)
            zero_b = big.tile([68, 1280], FP)
            nc.vector.memset(zero_b[:], 0.0)
            for i in range(3):
                nc.sync.dma_start(out=bstage[i][:], in_=zero_b[:])

            # ---- stage A: y_down [128ch, 34r, 66c] = w_down . x + b_down ----
            row_blocks = [(0, 6), (6, 12), (12, 18), (18, 24), (24, 30), (30, 34)]
            for bi, (r0, r1) in enumerate(row_blocks):
                nr = r1 - r0
                pa = psAZ.tile([128, 6, 66], FP, tag="AZ")
                nc.tensor.matmul(
                    pa[:, 0:nr, :], wdt[:, 0, :], xa[:, 1 + r0 : 1 + r1, 1:67],
                    start=True, stop=False,
                )
                nc.tensor.matmul(
                    pa[:, 0:nr, :], wdt[:, 1, :], xb[:, 1 + r0 : 1 + r1, 1:67],
                    start=False, stop=True,
                )
                eng = nc.vector if bi % 2 == 0 else nc.scalar
                if r0 == 0:
                    nc.vector.tensor_scalar(
                        ydown[:, 0:1, :], pa[:, 0:1, :], bd[:], edge[:, 0:1],
                        op0=ALU.add, op1=ALU.mult,
                    )
                    nc.scalar.add(ydown[:, 1:6, :], pa[:, 1:6, :], add=bd[:])
                elif r1 == 34:
                    nc.vector.tensor_scalar(
                        ydown[:, 33:34, :], pa[:, 3:4, :], bd[:], edge[:, 1:2],
                        op0=ALU.add, op1=ALU.mult,
                    )
                    nc.scalar.add(ydown[:, 30:33, :], pa[:, 0:3, :], add=bd[:])
                else:
                    if bi % 2 == 0:
                        nc.vector.tensor_scalar(
                            ydown[:, r0:r1, :], pa[:, 0:nr, :], bd[:], None,
                            op0=ALU.add,
                        )
                    else:
                        nc.scalar.add(ydown[:, r0:r1, :], pa[:, 0:nr, :], add=bd[:])
            # zero the w=-1 / w=64 columns (conv zero-padding semantics)
            nc.vector.memset(ydown[:, :, 0:1], 0.0)
            nc.vector.memset(ydown[:, :, 65:66], 0.0)

            # ---- stage Z: zT [68col, 36r, 256ch] = (w_out . x)^T ----
            for g in range(9):
                pz = psAZ.tile([68, 4, 256], FP, tag="AZ")
                for rr in range(4):
                    r = 4 * g + rr
                    nc.tensor.matmul(
                        pz[:, rr, :], xa[:, r, :], wot[:, 0, :], start=True, stop=False
                    )
                    nc.tensor.matmul(
                        pz[:, rr, :], xb[:, r, :], wot[:, 1, :], start=False, stop=True
                    )
                if g % 2 == 0:
                    nc.vector.tensor_copy(zt[:, 4 * g : 4 * g + 4, :], pz[:])
                else:
                    nc.scalar.copy(zt[:, 4 * g : 4 * g + 4, :], pz[:])

            # ---- stage B: enc -> exp(enc + b_enc) [100, 32, 64] ----
            for b4 in range(4):
                pb = psB.tile([100, 8, 64], FP, tag="B")
                k = 0
                for di in range(3):
                    for dj in range(3):
                        nc.tensor.matmul(
                            pb[:],
                            wet[:, 3 * di + dj, :],
                            ydown[:, di + 8 * b4 : di + 8 * b4 + 8, dj : dj + 64],
                            start=(k == 0), stop=(k == 8),
                        )
                        k += 1
                nc.scalar.activation(
                    expv[:, 8 * b4 : 8 * b4 + 8, :], pb[:], AF.Exp, bias=be[:]
                )

            # ---- stage C: transpose + group sums + normalize -> maskv ----
            expf = expv[:].rearrange("p a b -> p (a b)")
            for kchunk in range(16):
                pc = psCD.tile([128, 256], FP, tag="CD")
                nc.tensor.matmul(
                    pc[:, 0:104],
                    expf[:, 128 * kchunk : 128 * (kchunk + 1)],
                    saug[:],
                    start=True, stop=True,
                )
                nc.vector.reciprocal(inv[:, kchunk, :], pc[:, 100:104])
                inv_b = _ap(inv[:], kchunk * 4, [[64, 128], [0, 25], [1, 4]])
                nc.vector.tensor_tensor(
                    maskv[:, kchunk, :].rearrange("p (k q) -> p k q", q=4),
                    pc[:, 0:100].rearrange("p (k q) -> p k q", q=4),
                    inv_b,
                    op=ALU.mult,
                )

            # ---- stage D: banded reassembly + b_out ----
            for h in range(32):
                slot = h % 3
                bt = bpool.tile([68, 1280], FP, tag=f"bt{slot}")
                stg = bstage[slot]
                src = maskv[(h % 2) * 64 : (h % 2) * 64 + 64, h // 2, :]
                dst = _ap(stg[:], 0, [[1300, 64], [1280, 5], [1, 20]])
                seng = nc.gpsimd if h % 2 == 0 else nc.sync
                seng.dma_start(out=dst, in_=src)
                reng = nc.scalar if h % 2 == 0 else nc.sync
                reng.dma_start(out=bt[:], in_=stg[:])

                for half in range(2):
                    pd = psCD.tile([128, 256], FP, tag="CD")
                    for i in range(5):
                        rhs = _ap(bt[:], 4 * i, [[1280, 68], [20, 64], [1, 4]])
                        nc.tensor.matmul(
                            pd[:].rearrange("p (w q) -> p w q", q=4),
                            zt[:, h + i, 128 * half : 128 * half + 128],
                            rhs,
                            start=(i == 0), stop=(i == 4),
                        )
                    ob = opool.tile([128, 2, 64, 2], FP, tag="ob")
                    pd_v = _ap(pd[:], 0, [[256, 128], [2, 2], [4, 64], [1, 2]])
                    if half == 0:
                        nc.vector.tensor_scalar(
                            ob[:], pd_v, bo[:, 0:1], None, op0=ALU.add
                        )
                    else:
                        nc.scalar.add(ob[:], pd_v, add=bo[:, 1:2])
                    oeng = nc.sync if half == 0 else nc.scalar
                    oeng.dma_start(
                        out=out_d[128 * half : 128 * (half + 1), 2 * h : 2 * h + 2, :],
                        in_=ob[:].rearrange("p a w q -> p a (w q)"),
                    )

    nc.compile()
    return nc


def _host_prep(x, w_down, b_down, w_enc, b_enc, w_out, b_out):
    x = np.asarray(x, np.float32)
    xp = np.pad(x, [(0, 0), (0, 0), (2, 2), (2, 2)])
    wdt = np.ascontiguousarray(np.asarray(w_down, np.float32)[:, :, 0, 0].T)
    wet = np.ascontiguousarray(
        np.asarray(w_enc, np.float32).transpose(1, 2, 3, 0).reshape(128, 9, 100)
    )
    wot = np.ascontiguousarray(np.asarray(w_out, np.float32)[:, :, 0, 0].T)
    bd = np.asarray(b_down, np.float32).reshape(128, 1)
    be = np.asarray(b_enc, np.float32).reshape(100, 1)
    bo = np.asarray(b_out, np.float32).reshape(256, 1)
    # saug: permuted identity (e=(i5,j5,p4) -> e'=(j5,i5,p4)) + 4 group-sum cols
    saug = np.zeros((100, 104), np.float32)
    for i in range(5):
        for j in range(5):
            for p in range(4):
                saug[(i * 5 + j) * 4 + p, j * 20 + i * 4 + p] = 1.0
    for e in range(100):
        saug[e, 100 + e % 4] = 1.0
    in_maps = []
    for c in range(NCORES):
        n, hh = c // 2, c % 2
        xs = np.ascontiguousarray(xp[n, :, hh * 32 : hh * 32 + 36, :])
        edge = np.array(
            [[0.0 if hh == 0 else 1.0, 0.0 if hh == 1 else 1.0]], np.float32
        )
        in_maps.append(
            dict(xs=xs, wdt=wdt, wet=wet, wot=wot, bd=bd, be=be, bo=bo,
                 saug=saug, edge=edge)
        )
    return in_maps


last_exec_time_ns = None


def kernel(x, w_down, b_down, w_enc, b_enc, w_out, b_out):
    global last_exec_time_ns
    nc = _build()
    in_maps = _host_prep(x, w_down, b_down, w_enc, b_enc, w_out, b_out)
    res = run_bass_kernel_spmd(nc, in_maps, list(range(NCORES)))
    last_exec_time_ns = res.exec_time_ns
    out = np.empty((4, 256, 128, 128), np.float32)
    for c in range(NCORES):
        n, hh = c // 2, c % 2
        out[n, :, hh * 64 : (hh + 1) * 64, :] = res.results[c]["out"]
    return out


# revision 10
# speedup vs baseline: 2.2853x; 2.2853x over previous
"""CARAFE content-aware upsampling on 8 Trainium2 NeuronCores.

Strategy (data parallel, hint-compliant):
  8 cores = 4 batch images x 2 row-halves (32 low-res rows each, +2-row halo).
  Per core, fully fused pipeline in SBUF:
    A) y_down = conv1x1(x, w_down)+b_down        (PE, K=256 in 2 chunks)
    Z) zT = (w_out . x) transposed               (PE produces [col, ch] directly)
    B) enc = conv3x3(y_down, w_enc)              (PE, 9 shifted accum matmuls)
    C) mask = softmax over 25 taps (4 groups)    (PE transpose+group-sums via an
       augmented selector matmul, DVE reciprocal + normalize)
    D) out = sum_k zT[window] * mask  + b_out    (PE: per-row banded matmuls;
       banded mask matrix built by a DRAM-roundtrip diagonal scatter DMA)
  The final 1x1 conv (w_out) is folded BEFORE reassembly (z-trick): conv and
  reassembly commute since both are linear; this runs the big conv at low res
  and skips materializing the upsampled intermediate.

Layouts:
  xs     [256, 36, 68]  zero-padded shard (rows h0-2..h1+2, cols -2..65)
  zT     [68, 36, 256]  col-on-partition transpose of z = w_out . x
  B_h    [68, 1280]     banded masks: B[w+j, w*20 + i*4 + p] = mask[h,w,i,j,p]
  out    [256, 64, 128] hi-res shard
"""

import sys
import functools
import numpy as np
from contextlib import ExitStack

for _p in ("/opt/trn_rl_repo",):
    if _p not in sys.path:
        sys.path.insert(0, _p)

import concourse.bass as bass
import concourse.bacc as bacc
import concourse.mybir as mybir
import concourse.tile as tile
from concourse.bass_utils import run_bass_kernel_spmd

NCORES = 8
FP = mybir.dt.float32
BF = mybir.dt.bfloat16
AF = mybir.ActivationFunctionType
ALU = mybir.AluOpType


def _ap(base, offset_delta, dims):
    return bass.AP(tensor=base.tensor, offset=base.offset + offset_delta, ap=dims)


@functools.lru_cache(maxsize=1)
def _build():
    nc = bacc.Bacc("TRN2", target_bir_lowering=False, debug=False, num_devices=NCORES)

    xs_d = nc.declare_dram_parameter("xs", [256, 36, 68], BF, isOutput=False)
    wdt_d = nc.declare_dram_parameter("wdt", [256, 128], BF, isOutput=False)
    wet_d = nc.declare_dram_parameter("wet", [128, 9, 100], BF, isOutput=False)
    wot_d = nc.declare_dram_parameter("wot", [256, 256], BF, isOutput=False)
    bd_d = nc.declare_dram_parameter("bd", [128, 1], FP, isOutput=False)
    be_d = nc.declare_dram_parameter("be", [100, 1], FP, isOutput=False)
    bo_d = nc.declare_dram_parameter("bo", [256, 1], FP, isOutput=False)
    saug_d = nc.declare_dram_parameter("saug", [100, 104], BF, isOutput=False)
    edge_d = nc.declare_dram_parameter("edge", [1, 2], FP, isOutput=False)
    out_d = nc.declare_dram_parameter("out", [256, 64, 128], FP, isOutput=True)

    with tile.TileContext(nc) as tc:
        with ExitStack() as ctx:
            const = ctx.enter_context(tc.tile_pool(name="const", bufs=1))
            big = ctx.enter_context(tc.tile_pool(name="big", bufs=1))
            bpool = ctx.enter_context(tc.tile_pool(name="bpool", bufs=1))
            opool = ctx.enter_context(tc.tile_pool(name="opool", bufs=3))
            dpool = ctx.enter_context(tc.tile_pool(name="dpool", bufs=1, space="DRAM"))
            psAZ = ctx.enter_context(tc.tile_pool(name="psAZ", bufs=2, space="PSUM"))
            psB = ctx.enter_context(tc.tile_pool(name="psB", bufs=2, space="PSUM"))
            psCD = ctx.enter_context(tc.tile_pool(name="psCD", bufs=2, space="PSUM"))

            # ---- loads ----
            xa = big.tile([128, 36, 68], BF)
            xb = big.tile([128, 36, 68], BF)
            nc.sync.dma_start(out=xa[:], in_=xs_d[0:128])
            nc.sync.dma_start(out=xb[:], in_=xs_d[128:256])
            wdt = const.tile([128, 2, 128], BF)
            nc.sync.dma_start(out=wdt[:, 0, :], in_=wdt_d[0:128])
            nc.sync.dma_start(out=wdt[:, 1, :], in_=wdt_d[128:256])
            wet = const.tile([128, 9, 100], BF)
            nc.sync.dma_start(out=wet[:], in_=wet_d[:])
            wot = const.tile([128, 2, 256], BF)
            nc.sync.dma_start(out=wot[:, 0, :], in_=wot_d[0:128])
            nc.sync.dma_start(out=wot[:, 1, :], in_=wot_d[128:256])
            bd = const.tile([128, 1], FP)
            nc.sync.dma_start(out=bd[:], in_=bd_d[:])
            be = const.tile([100, 1], FP)
            nc.sync.dma_start(out=be[:], in_=be_d[:])
            bo = const.tile([128, 2], FP)
            nc.sync.dma_start(out=bo[:, 0:1], in_=bo_d[0:128])
            nc.sync.dma_start(out=bo[:, 1:2], in_=bo_d[128:256])
            saug = const.tile([100, 104], BF)
            nc.sync.dma_start(out=saug[:], in_=saug_d[:])
            edge = const.tile([128, 2], FP)
            nc.sync.dma_start(
                out=edge[:],
                in_=bass.AP(tensor=edge_d, offset=0, ap=[[0, 128], [1, 2]]),
            )

            ydown = big.tile([128, 34, 66], BF)
            zt = big.tile([68, 36, 256], BF)
            expv = big.tile([100, 32, 64], BF)
            maskv = big.tile([128, 16, 100], BF)
            inv = big.tile([128, 16, 4], FP)

            # DRAM staging for the banded-mask scatter (4 rotating slots,
            # contiguous so reloads can batch adjacent pairs).
            bstage_all = dpool.tile([4, 68, 1280], BF, name="bstage_all")
            zero_b = big.tile([68, 1280], BF)
            nc.vector.memset(zero_b[:], 0.0)
            for i in range(4):
                nc.sync.dma_start(out=bstage_all[i], in_=zero_b[:])

            # ---- stage A: y_down [128ch, 34r, 66c] = w_down . x + b_down ----
            row_blocks = [(0, 6), (6, 12), (12, 18), (18, 24), (24, 30), (30, 34)]
            for bi, (r0, r1) in enumerate(row_blocks):
                nr = r1 - r0
                pa = psAZ.tile([128, 6, 66], FP, tag="AZ")
                nc.tensor.matmul(
                    pa[:, 0:nr, :], wdt[:, 0, :], xa[:, 1 + r0 : 1 + r1, 1:67],
                    start=True, stop=False,
                )
                nc.tensor.matmul(
                    pa[:, 0:nr, :], wdt[:, 1, :], xb[:, 1 + r0 : 1 + r1, 1:67],
                    start=False, stop=True,
                )
                eng = nc.vector if bi % 2 == 0 else nc.scalar
                if r0 == 0:
                    nc.vector.tensor_scalar(
                        ydown[:, 0:1, :], pa[:, 0:1, :], bd[:], edge[:, 0:1],
                        op0=ALU.add, op1=ALU.mult,
                    )
                    nc.scalar.add(ydown[:, 1:6, :], pa[:, 1:6, :], add=bd[:])
                elif r1 == 34:
                    nc.vector.tensor_scalar(
                        ydown[:, 33:34, :], pa[:, 3:4, :], bd[:], edge[:, 1:2],
                        op0=ALU.add, op1=ALU.mult,
                    )
                    nc.scalar.add(ydown[:, 30:33, :], pa[:, 0:3, :], add=bd[:])
                else:
                    if bi % 2 == 0:
                        nc.vector.tensor_scalar(
                            ydown[:, r0:r1, :], pa[:, 0:nr, :], bd[:], None,
                            op0=ALU.add,
                        )
                    else:
                        nc.scalar.add(ydown[:, r0:r1, :], pa[:, 0:nr, :], add=bd[:])
            # zero the w=-1 / w=64 columns (conv zero-padding semantics)
            nc.vector.memset(ydown[:, :, 0:1], 0.0)
            nc.vector.memset(ydown[:, :, 65:66], 0.0)

            # ---- stage Z: zT [68col, 36r, 256ch] = (w_out . x)^T ----
            for g in range(9):
                pz = psAZ.tile([68, 4, 256], FP, tag="AZ")
                for rr in range(4):
                    r = 4 * g + rr
                    nc.tensor.matmul(
                        pz[:, rr, :], xa[:, r, :], wot[:, 0, :], start=True, stop=False
                    )
                    nc.tensor.matmul(
                        pz[:, rr, :], xb[:, r, :], wot[:, 1, :], start=False, stop=True
                    )
                if g % 2 == 0:
                    nc.vector.tensor_copy(zt[:, 4 * g : 4 * g + 4, :], pz[:])
                else:
                    nc.scalar.copy(zt[:, 4 * g : 4 * g + 4, :], pz[:])

            # ---- stage B: enc -> exp(enc + b_enc) [100, 32, 64] ----
            for b4 in range(4):
                pb = psB.tile([100, 8, 64], FP, tag="B")
                k = 0
                for di in range(3):
                    for dj in range(3):
                        nc.tensor.matmul(
                            pb[:],
                            wet[:, 3 * di + dj, :],
                            ydown[:, di + 8 * b4 : di + 8 * b4 + 8, dj : dj + 64],
                            start=(k == 0), stop=(k == 8),
                        )
                        k += 1
                nc.scalar.activation(
                    expv[:, 8 * b4 : 8 * b4 + 8, :], pb[:], AF.Exp, bias=be[:]
                )

            # ---- stage C: transpose + group sums + normalize -> maskv ----
            expf = expv[:].rearrange("p a b -> p (a b)")
            for kchunk in range(16):
                pc = psCD.tile([128, 256], FP, tag="CD")
                nc.tensor.matmul(
                    pc[:, 0:104],
                    expf[:, 128 * kchunk : 128 * (kchunk + 1)],
                    saug[:],
                    start=True, stop=True,
                )
                nc.vector.reciprocal(inv[:, kchunk, :], pc[:, 100:104])
                inv_b = _ap(inv[:], kchunk * 4, [[64, 128], [0, 25], [1, 4]])
                nc.vector.tensor_tensor(
                    maskv[:, kchunk, :].rearrange("p (k q) -> p k q", q=4),
                    pc[:, 0:100].rearrange("p (k q) -> p k q", q=4),
                    inv_b,
                    op=ALU.mult,
                )

            # ---- stage D: banded reassembly + b_out ----
            # Paired rows: 2 scatters -> 1 batched reload; out rows batched 4h.
            obs = [None, None]
            for hp in range(16):
                bt2 = bpool.tile([68, 2, 1280], BF, tag=f"bt{hp % 2}")
                for hh in range(2):
                    h = 2 * hp + hh
                    slot = h % 4
                    src = maskv[(h % 2) * 64 : (h % 2) * 64 + 64, h // 2, :]
                    dst = _ap(
                        bstage_all[:], slot * 68 * 1280, [[1300, 64], [1280, 5], [1, 20]]
                    )
                    seng = nc.gpsimd if h % 2 == 0 else nc.sync
                    seng.dma_start(out=dst, in_=src)
                # batched reload of both staged rows (adjacent dram slots)
                src2 = _ap(
                    bstage_all[:],
                    ((2 * hp) % 4) * 68 * 1280,
                    [[1280, 68], [68 * 1280, 2], [1, 1280]],
                )
                reng = nc.sync if hp % 2 == 0 else nc.scalar
                reng.dma_start(out=bt2[:], in_=src2)

                for hh in range(2):
                    h = 2 * hp + hh
                    if h % 4 == 0:
                        obs[0] = opool.tile([128, 8, 64, 2], FP, tag="ob0", name="ob0")
                        obs[1] = opool.tile([128, 8, 64, 2], FP, tag="ob1", name="ob1")
                    for half in range(2):
                        pd = psCD.tile([128, 256], FP, tag="CD")
                        for i in range(5):
                            rhs = _ap(
                                bt2[:], hh * 1280 + 4 * i, [[2560, 68], [20, 64], [1, 4]]
                            )
                            nc.tensor.matmul(
                                pd[:].rearrange("p (w q) -> p w q", q=4),
                                zt[:, h + i, 128 * half : 128 * half + 128],
                                rhs,
                                start=(i == 0), stop=(i == 4),
                            )
                        ob = obs[half]
                        q = h % 4
                        pd_v = _ap(pd[:], 0, [[256, 128], [2, 2], [4, 64], [1, 2]])
                        if half == 0:
                            nc.vector.tensor_scalar(
                                ob[:, 2 * q : 2 * q + 2], pd_v, bo[:, 0:1], None,
                                op0=ALU.add,
                            )
                        else:
                            nc.scalar.add(ob[:, 2 * q : 2 * q + 2], pd_v, add=bo[:, 1:2])
                    if h % 4 == 3:
                        for half in range(2):
                            oeng = nc.sync if half == 0 else nc.scalar
                            oeng.dma_start(
                                out=out_d[
                                    128 * half : 128 * (half + 1),
                                    2 * h - 6 : 2 * h + 2,
                                    :,
                                ],
                                in_=obs[half][:].rearrange("p a w q -> p a (w q)"),
                            )

    nc.compile()
    return nc


def _host_prep(x, w_down, b_down, w_enc, b_enc, w_out, b_out):
    import ml_dtypes

    bft = ml_dtypes.bfloat16
    x = np.asarray(x, np.float32)
    xp = np.pad(x, [(0, 0), (0, 0), (2, 2), (2, 2)]).astype(bft)
    wdt = np.ascontiguousarray(np.asarray(w_down, np.float32)[:, :, 0, 0].T.astype(bft))
    wet = np.ascontiguousarray(
        np.asarray(w_enc, np.float32).transpose(1, 2, 3, 0).reshape(128, 9, 100)
    ).astype(bft)
    wot = np.ascontiguousarray(np.asarray(w_out, np.float32)[:, :, 0, 0].T.astype(bft))
    bd = np.asarray(b_down, np.float32).reshape(128, 1)
    be = np.asarray(b_enc, np.float32).reshape(100, 1)
    bo = np.asarray(b_out, np.float32).reshape(256, 1)
    # saug: permuted identity (e=(i5,j5,p4) -> e'=(j5,i5,p4)) + 4 group-sum cols
    saug = np.zeros((100, 104), bft)
    for i in range(5):
        for j in range(5):
            for p in range(4):
                saug[(i * 5 + j) * 4 + p, j * 20 + i * 4 + p] = 1.0
    for e in range(100):
        saug[e, 100 + e % 4] = 1.0
    in_maps = []
    for c in range(NCORES):
        n, hh = c // 2, c % 2
        xs = np.ascontiguousarray(xp[n, :, hh * 32 : hh * 32 + 36, :])
        edge = np.array(
            [[0.0 if hh == 0 else 1.0, 0.0 if hh == 1 else 1.0]], np.float32
        )
        in_maps.append(
            dict(xs=xs, wdt=wdt, wet=wet, wot=wot, bd=bd, be=be, bo=bo,
                 saug=saug, edge=edge)
        )
    return in_maps


last_exec_time_ns = None


def kernel(x, w_down, b_down, w_enc, b_enc, w_out, b_out):
    global last_exec_time_ns
    nc = _build()
    in_maps = _host_prep(x, w_down, b_down, w_enc, b_enc, w_out, b_out)
    res = run_bass_kernel_spmd(nc, in_maps, list(range(NCORES)))
    last_exec_time_ns = res.exec_time_ns
    out = np.empty((4, 256, 128, 128), np.float32)
    for c in range(NCORES):
        n, hh = c // 2, c % 2
        out[n, :, hh * 64 : (hh + 1) * 64, :] = res.results[c]["out"]
    return out


# revision 12
# speedup vs baseline: 2.4551x; 1.0743x over previous
"""CARAFE content-aware upsampling on 8 Trainium2 NeuronCores.

Strategy (data parallel, hint-compliant):
  8 cores = 4 batch images x 2 row-halves (32 low-res rows each, +2-row halo).
  Per core, fully fused pipeline in SBUF:
    A) y_down = conv1x1(x, w_down)+b_down        (PE, K=256 in 2 chunks)
    Z) zT = (w_out . x) transposed               (PE produces [col, ch] directly)
    B) enc = conv3x3(y_down, w_enc)              (PE, 9 shifted accum matmuls)
    C) mask = softmax over 25 taps (4 groups)    (PE transpose+group-sums via an
       augmented selector matmul, DVE reciprocal + normalize)
    D) out = sum_k zT[window] * mask  + b_out    (PE: per-row banded matmuls;
       banded mask matrix built by a DRAM-roundtrip diagonal scatter DMA)
  The final 1x1 conv (w_out) is folded BEFORE reassembly (z-trick): conv and
  reassembly commute since both are linear; this runs the big conv at low res
  and skips materializing the upsampled intermediate.

Layouts:
  xs     [256, 36, 68]  zero-padded shard (rows h0-2..h1+2, cols -2..65)
  zT     [68, 36, 256]  col-on-partition transpose of z = w_out . x
  B_h    [68, 1280]     banded masks: B[w+j, w*20 + i*4 + p] = mask[h,w,i,j,p]
  out    [256, 64, 128] hi-res shard
"""

import sys
import functools
import numpy as np
from contextlib import ExitStack

for _p in ("/opt/trn_rl_repo",):
    if _p not in sys.path:
        sys.path.insert(0, _p)

import concourse.bass as bass
import concourse.bacc as bacc
import concourse.mybir as mybir
import concourse.tile as tile
from concourse.bass_utils import run_bass_kernel_spmd

NCORES = 8
FP = mybir.dt.float32
BF = mybir.dt.bfloat16
AF = mybir.ActivationFunctionType
ALU = mybir.AluOpType


def _ap(base, offset_delta, dims):
    return bass.AP(tensor=base.tensor, offset=base.offset + offset_delta, ap=dims)


@functools.lru_cache(maxsize=1)
def _build():
    nc = bacc.Bacc("TRN2", target_bir_lowering=False, debug=False, num_devices=NCORES)

    xs_d = nc.declare_dram_parameter("xs", [256, 36, 68], BF, isOutput=False)
    wdt_d = nc.declare_dram_parameter("wdt", [256, 128], BF, isOutput=False)
    wet_d = nc.declare_dram_parameter("wet", [128, 9, 100], BF, isOutput=False)
    wot_d = nc.declare_dram_parameter("wot", [256, 256], BF, isOutput=False)
    bd_d = nc.declare_dram_parameter("bd", [128, 1], FP, isOutput=False)
    be_d = nc.declare_dram_parameter("be", [100, 1], FP, isOutput=False)
    bo_d = nc.declare_dram_parameter("bo", [256, 1], FP, isOutput=False)
    saug_d = nc.declare_dram_parameter("saug", [100, 104], BF, isOutput=False)
    edge_d = nc.declare_dram_parameter("edge", [1, 2], FP, isOutput=False)
    out_d = nc.declare_dram_parameter("out", [256, 64, 128], FP, isOutput=True)

    with tile.TileContext(nc) as tc:
        with ExitStack() as ctx:
            const = ctx.enter_context(tc.tile_pool(name="const", bufs=1))
            big = ctx.enter_context(tc.tile_pool(name="big", bufs=1))
            bpool = ctx.enter_context(tc.tile_pool(name="bpool", bufs=1))
            opool = ctx.enter_context(tc.tile_pool(name="opool", bufs=3))
            dpool = ctx.enter_context(tc.tile_pool(name="dpool", bufs=1, space="DRAM"))

            # ---- loads ----
            xa = big.tile([128, 36, 68], BF)
            xb = big.tile([128, 36, 68], BF)
            nc.sync.dma_start(out=xa[:], in_=xs_d[0:128])
            nc.sync.dma_start(out=xb[:], in_=xs_d[128:256])
            wdt = const.tile([128, 2, 128], BF)
            nc.sync.dma_start(out=wdt[:, 0, :], in_=wdt_d[0:128])
            nc.sync.dma_start(out=wdt[:, 1, :], in_=wdt_d[128:256])
            wet = const.tile([128, 9, 100], BF)
            nc.sync.dma_start(out=wet[:], in_=wet_d[:])
            wot = const.tile([128, 2, 256], BF)
            nc.sync.dma_start(out=wot[:, 0, :], in_=wot_d[0:128])
            nc.sync.dma_start(out=wot[:, 1, :], in_=wot_d[128:256])
            bd = const.tile([128, 1], FP)
            nc.sync.dma_start(out=bd[:], in_=bd_d[:])
            be = const.tile([100, 1], FP)
            nc.sync.dma_start(out=be[:], in_=be_d[:])
            bo = const.tile([128, 2], FP)
            nc.sync.dma_start(out=bo[:, 0:1], in_=bo_d[0:128])
            nc.sync.dma_start(out=bo[:, 1:2], in_=bo_d[128:256])
            saug = const.tile([100, 104], BF)
            nc.sync.dma_start(out=saug[:], in_=saug_d[:])
            edge = const.tile([128, 2], FP)
            nc.sync.dma_start(
                out=edge[:],
                in_=bass.AP(tensor=edge_d, offset=0, ap=[[0, 128], [1, 2]]),
            )

            ydown = big.tile([128, 34, 66], BF)
            zt = big.tile([68, 36, 256], BF)
            expv = big.tile([100, 32, 64], BF)
            maskv = big.tile([128, 16, 100], BF)
            inv = big.tile([128, 16, 4], FP)

            # DRAM staging for the banded-mask scatter (8 rotating slots,
            # contiguous so reloads can batch adjacent pairs).
            bstage_all = dpool.tile([8, 68, 1280], BF, name="bstage_all")
            zero_b = big.tile([68, 1280], BF)
            nc.vector.memset(zero_b[:], 0.0)
            for i in range(8):
                nc.sync.dma_start(out=bstage_all[i], in_=zero_b[:])
            # all 32 banded-mask rows live in one SBUF tile; reloads fill
            # disjoint slices so the whole scatter pipeline runs ahead of PE
            btall = big.tile([68, 32, 1280], BF)

            psum_ab = ExitStack()
            psAZ = psum_ab.enter_context(tc.tile_pool(name="psAZ", bufs=2, space="PSUM"))
            psB = psum_ab.enter_context(tc.tile_pool(name="psB", bufs=2, space="PSUM"))
            # ---- stage A: y_down [128ch, 34r, 66c] = w_down . x + b_down ----
            row_blocks = [(0, 6), (6, 12), (12, 18), (18, 24), (24, 30), (30, 34)]
            for bi, (r0, r1) in enumerate(row_blocks):
                nr = r1 - r0
                pa = psAZ.tile([128, 6, 66], FP, tag="AZ")
                nc.tensor.matmul(
                    pa[:, 0:nr, :], wdt[:, 0, :], xa[:, 1 + r0 : 1 + r1, 1:67],
                    start=True, stop=False,
                )
                nc.tensor.matmul(
                    pa[:, 0:nr, :], wdt[:, 1, :], xb[:, 1 + r0 : 1 + r1, 1:67],
                    start=False, stop=True,
                )
                eng = nc.vector if bi % 2 == 0 else nc.scalar
                if r0 == 0:
                    nc.vector.tensor_scalar(
                        ydown[:, 0:1, :], pa[:, 0:1, :], bd[:], edge[:, 0:1],
                        op0=ALU.add, op1=ALU.mult,
                    )
                    nc.scalar.add(ydown[:, 1:6, :], pa[:, 1:6, :], add=bd[:])
                elif r1 == 34:
                    nc.vector.tensor_scalar(
                        ydown[:, 33:34, :], pa[:, 3:4, :], bd[:], edge[:, 1:2],
                        op0=ALU.add, op1=ALU.mult,
                    )
                    nc.scalar.add(ydown[:, 30:33, :], pa[:, 0:3, :], add=bd[:])
                else:
                    if bi % 2 == 0:
                        nc.vector.tensor_scalar(
                            ydown[:, r0:r1, :], pa[:, 0:nr, :], bd[:], None,
                            op0=ALU.add,
                        )
                    else:
                        nc.scalar.add(ydown[:, r0:r1, :], pa[:, 0:nr, :], add=bd[:])
            # zero the w=-1 / w=64 columns (conv zero-padding semantics)
            nc.vector.memset(ydown[:, :, 0:1], 0.0)
            nc.vector.memset(ydown[:, :, 65:66], 0.0)

            # ---- stage Z: zT [68col, 36r, 256ch] = (w_out . x)^T ----
            for g in range(9):
                pz = psAZ.tile([68, 4, 256], FP, tag="AZ")
                for rr in range(4):
                    r = 4 * g + rr
                    nc.tensor.matmul(
                        pz[:, rr, :], xa[:, r, :], wot[:, 0, :], start=True, stop=False
                    )
                    nc.tensor.matmul(
                        pz[:, rr, :], xb[:, r, :], wot[:, 1, :], start=False, stop=True
                    )
                if g % 2 == 0:
                    nc.vector.tensor_copy(zt[:, 4 * g : 4 * g + 4, :], pz[:])
                else:
                    nc.scalar.copy(zt[:, 4 * g : 4 * g + 4, :], pz[:])

            # ---- stage B: enc -> exp(enc + b_enc) [100, 32, 64] ----
            for b4 in range(4):
                pb = psB.tile([100, 8, 64], FP, tag="B")
                k = 0
                for di in range(3):
                    for dj in range(3):
                        nc.tensor.matmul(
                            pb[:],
                            wet[:, 3 * di + dj, :],
                            ydown[:, di + 8 * b4 : di + 8 * b4 + 8, dj : dj + 64],
                            start=(k == 0), stop=(k == 8),
                        )
                        k += 1
                nc.scalar.activation(
                    expv[:, 8 * b4 : 8 * b4 + 8, :], pb[:], AF.Exp, bias=be[:]
                )

            # close A/Z/B psum pools to free banks for C/D
            psum_ab.close()
            psC = ctx.enter_context(tc.tile_pool(name="psC", bufs=2, space="PSUM"))
            psD = ctx.enter_context(tc.tile_pool(name="psD", bufs=4, space="PSUM"))

            # ---- stage C: transpose + group sums + normalize -> maskv ----
            # interleaved with the banded-mask scatter/reload pipeline so all
            # DMA staging runs ahead of the stage-D matmul stream
            expf = expv[:].rearrange("p a b -> p (a b)")
            for kc in range(16):
                pc = psC.tile([128, 104], FP, tag="C")
                nc.tensor.matmul(
                    pc[:],
                    expf[:, 128 * kc : 128 * (kc + 1)],
                    saug[:],
                    start=True, stop=True,
                )
                nc.vector.reciprocal(inv[:, kc, :], pc[:, 100:104])
                inv_b = _ap(inv[:], kc * 4, [[64, 128], [0, 25], [1, 4]])
                nc.vector.tensor_tensor(
                    maskv[:, kc, :].rearrange("p (k q) -> p k q", q=4),
                    pc[:, 0:100].rearrange("p (k q) -> p k q", q=4),
                    inv_b,
                    op=ALU.mult,
                )
                # scatter the two rows of this chunk, then batch-reload them
                for hh in range(2):
                    h = 2 * kc + hh
                    slot = h % 8
                    srcm = maskv[hh * 64 : hh * 64 + 64, kc, :]
                    dstm = _ap(
                        bstage_all[:], slot * 68 * 1280,
                        [[1300, 64], [1280, 5], [1, 20]],
                    )
                    seng = nc.gpsimd if hh == 0 else nc.sync
                    seng.dma_start(out=dstm, in_=srcm)
                src2 = _ap(
                    bstage_all[:],
                    ((2 * kc) % 8) * 68 * 1280,
                    [[1280, 68], [68 * 1280, 2], [1, 1280]],
                )
                reng = nc.sync if kc % 2 == 0 else nc.scalar
                reng.dma_start(out=btall[:, 2 * kc : 2 * kc + 2, :], in_=src2)

            # ---- stage D: banded reassembly + b_out ----
            obs = [None, None]
            for h in range(32):
                if h % 4 == 0:
                    obs[0] = opool.tile([128, 8, 64, 2], FP, tag="ob0", name="ob0")
                    obs[1] = opool.tile([128, 8, 64, 2], FP, tag="ob1", name="ob1")
                for half in range(2):
                    pd = psD.tile([128, 256], FP, tag="D")
                    for i in range(5):
                        rhs = _ap(
                            btall[:], h * 1280 + 4 * i, [[40960, 68], [20, 64], [1, 4]]
                        )
                        nc.tensor.matmul(
                            pd[:].rearrange("p (w q) -> p w q", q=4),
                            zt[:, h + i, 128 * half : 128 * half + 128],
                            rhs,
                            start=(i == 0), stop=(i == 4),
                        )
                    ob = obs[half]
                    q = h % 4
                    pd_v = _ap(pd[:], 0, [[256, 128], [2, 2], [4, 64], [1, 2]])
                    if half == 0:
                        nc.vector.tensor_scalar(
                            ob[:, 2 * q : 2 * q + 2], pd_v, bo[:, 0:1], None,
                            op0=ALU.add,
                        )
                    else:
                        nc.scalar.add(ob[:, 2 * q : 2 * q + 2], pd_v, add=bo[:, 1:2])
                if h % 4 == 3:
                    for half in range(2):
                        oeng = nc.sync if half == 0 else nc.scalar
                        oeng.dma_start(
                            out=out_d[
                                128 * half : 128 * (half + 1),
                                2 * h - 6 : 2 * h + 2,
                                :,
                            ],
                            in_=obs[half][:].rearrange("p a w q -> p a (w q)"),
                        )

    nc.compile()
    return nc


def _host_prep(x, w_down, b_down, w_enc, b_enc, w_out, b_out):
    import ml_dtypes

    bft = ml_dtypes.bfloat16
    x = np.asarray(x, np.float32)
    xp = np.pad(x, [(0, 0), (0, 0), (2, 2), (2, 2)]).astype(bft)
    wdt = np.ascontiguousarray(np.asarray(w_down, np.float32)[:, :, 0, 0].T.astype(bft))
    wet = np.ascontiguousarray(
        np.asarray(w_enc, np.float32).transpose(1, 2, 3, 0).reshape(128, 9, 100)
    ).astype(bft)
    wot = np.ascontiguousarray(np.asarray(w_out, np.float32)[:, :, 0, 0].T.astype(bft))
    bd = np.asarray(b_down, np.float32).reshape(128, 1)
    be = np.asarray(b_enc, np.float32).reshape(100, 1)
    bo = np.asarray(b_out, np.float32).reshape(256, 1)
    # saug: permuted identity (e=(i5,j5,p4) -> e'=(j5,i5,p4)) + 4 group-sum cols
    saug = np.zeros((100, 104), bft)
    for i in range(5):
        for j in range(5):
            for p in range(4):
                saug[(i * 5 + j) * 4 + p, j * 20 + i * 4 + p] = 1.0
    for e in range(100):
        saug[e, 100 + e % 4] = 1.0
    in_maps = []
    for c in range(NCORES):
        n, hh = c // 2, c % 2
        xs = np.ascontiguousarray(xp[n, :, hh * 32 : hh * 32 + 36, :])
        edge = np.array(
            [[0.0 if hh == 0 else 1.0, 0.0 if hh == 1 else 1.0]], np.float32
        )
        in_maps.append(
            dict(xs=xs, wdt=wdt, wet=wet, wot=wot, bd=bd, be=be, bo=bo,
                 saug=saug, edge=edge)
        )
    return in_maps


last_exec_time_ns = None


def kernel(x, w_down, b_down, w_enc, b_enc, w_out, b_out):
    global last_exec_time_ns
    nc = _build()
    in_maps = _host_prep(x, w_down, b_down, w_enc, b_enc, w_out, b_out)
    res = run_bass_kernel_spmd(nc, in_maps, list(range(NCORES)))
    last_exec_time_ns = res.exec_time_ns
    out = np.empty((4, 256, 128, 128), np.float32)
    for c in range(NCORES):
        n, hh = c // 2, c % 2
        out[n, :, hh * 64 : (hh + 1) * 64, :] = res.results[c]["out"]
    return out
